# revision 6
# baseline (speedup 1.0000x reference)
"""BSplineKan layer kernel for 8 trn2 NeuronCores (steady-state opt).

Math: out[b,o] = w_b*sum_i silu(x[b,i]) + w_s*sum_{i,k} bases_k(x[b,i]) * P[o,i,k]
reformulated as 9 truncated-power feature plane matmuls per input element plus
a host-side bias (see fold_weights). The silu term depends only on x, so the
host computes it in numpy; the device does only the 9-plane contraction.

The kernel is POWER-bound, not schedule-bound: the PE clock is throttled by
data toggle rate (measured: same NEFF runs 62us/rep on zero data, 100+us on
random data). Design choices that matter are energy choices:
  - fp16 matmul operands (halves SBUF/xbus streaming energy; fp32r ldw-opt
    is unnecessary since FWL is compiler-automatic for fp16). Rel err 7e-4.
  - WEIGHTS ARE THE STATIONARY OPERAND, features the moving one: the moving
    stream is 4x the elements of the stationary load (512x128 vs 128x128 per
    MM), and the feature planes are ~59% exact zeros with small magnitudes,
    so streaming features instead of dense-random weights minimizes toggle
    power. Output comes out transposed (yT[o, b]); the host untransposes.
  - Feature production runs ONE WHOLE REP AHEAD of the matmuls (into a
    double-buffered [128, 4*9*512] fp16 tile), wrapping across the For_i
    barrier so the post-barrier PE head is zero. No warm-up matmuls.
  - Matmuls are o-tile-major: for each of 8 o-tiles, each weight tile
    (j, ot) is loaded once and reused for all 4 batch chunks (4x fewer
    stationary-load SBUF reads), accumulating into 4 psum banks; the two
    o-tile parities ping-pong the two 4-bank sets so drains (2 DVE + 2 ACT
    per o-tile) overlap the next o-tile's matmuls. One 0.5MB y DMA per
    o-tile. r intermediates are fp16 to halve the gate->square traffic.
  - Only the Square activation table is used (silu on host): no act-table
    switches in the loop.

Sharding: contraction split — core c owns i in [128c, 128c+128). Host sums
the 8 fp16 partials in fp64 and adds bias + w_b * silu.
"""

import numpy as np

import concourse.bass as bass
import concourse.bass_utils as _bu
import concourse.mybir as mybir
import concourse.tile as tile
from concourse import bacc
from concourse.bass_utils import run_bass_kernel_spmd

F32 = mybir.dt.float32
F16 = mybir.dt.float16
AF = mybir.ActivationFunctionType
ALU = mybir.AluOpType

B, I, O = 2048, 1024, 1024
N_CORES = 8
I_LOC = I // N_CORES       # 128 contraction rows per core
H = 2.25 / 15.0            # knot spacing 0.15
KNOTS = [j * H - 1.125 for j in range(8, 15)]   # interior knots in (0,1)
LEFT = KNOTS[:3]           # relu^2(c - x) knots
RIGHT = KNOTS[3:]          # relu^2(x - c) knots
N_PLANES = 9               # v, v^2, 3 left, 4 right
N_OT = O // 128            # 8 output tiles (one psum bank each)
NCH = 4                    # batch chunks per rep
BC = B // NCH              # 512 rows per chunk

# kept for test.py compatibility (fp16 needs no walrus ldw-opt flag).
_orig_run_command = _bu.run_command


def _run_command_ldwopt(argv, **kwargs):
    return _orig_run_command(argv, **kwargs)


def fold_weights(P: np.ndarray, w_s: float):
    """Fold spline parameters into per-plane weights.

    Returns W (N_PLANES, I, O) float16 and bias (O,) float64.
    """
    Pd = P.astype(np.float64)
    O_, I_, _ = P.shape
    Pz = np.zeros((O_, I_, 18))
    Pz[:, :, 5:13] = Pd[:, :, 5:13]
    G = np.zeros((O_, I_, 15))
    for j in range(5, 15):
        G[:, :, j] = (0.5 * Pz[:, :, j] - 1.5 * Pz[:, :, j - 1]
                      + 1.5 * Pz[:, :, j - 2] - 0.5 * Pz[:, :, j - 3])
    c = np.array([j * H - 1.125 for j in range(15)])
    inv_h2 = 1.0 / (H * H)
    A = (G[:, :, 5] + G[:, :, 6] + G[:, :, 7]) * inv_h2
    Bq = -2.0 * (c[5] * G[:, :, 5] + c[6] * G[:, :, 6] + c[7] * G[:, :, 7]) * inv_h2
    Cq = (c[5] ** 2 * G[:, :, 5] + c[6] ** 2 * G[:, :, 6] + c[7] ** 2 * G[:, :, 7]) * inv_h2
    D = [G[:, :, 8 + t] * inv_h2 for t in range(7)]
    left_w = []
    for t, cj in enumerate(LEFT):
        A += D[t]
        Bq += -2.0 * cj * D[t]
        Cq += cj * cj * D[t]
        left_w.append(-D[t])
    right_w = [D[3 + t] for t in range(4)]
    planes = [Bq + A, A] + left_w + right_w                   # each (O, I)
    bias = (Cq + 0.5 * Bq + 0.25 * A).sum(axis=1) * w_s       # (O,)
    W = np.empty((N_PLANES, I_, O_), np.float16)
    for p, pw in enumerate(planes):
        W[p] = (w_s * pw).T.astype(np.float16)
    return W, bias


def build_kernel(reps: int = 1, unroll: int = 1, loop_unroll: int = 2):
    """Per-core Bass kernel (SPMD across 8 cores, contraction-split).

    reps>1 wraps the body in a hardware loop, emitting the body
    loop_unroll times per iteration. unroll>1 emits the body N times with
    no loop (sim-only steady-state measurement).
    """
    nc = bacc.Bacc("TRN2", target_bir_lowering=False, debug=False,
                   num_devices=N_CORES)
    xT_d = nc.dram_tensor("xT", [I_LOC, B], F32, kind="ExternalInput")
    W_d = nc.dram_tensor("Wf", [N_PLANES * I_LOC, O], F16, kind="ExternalInput")
    yT_d = nc.dram_tensor("yT", [O, B], F16, kind="ExternalOutput")

    with tile.TileContext(nc) as tc:
        with (
            tc.tile_pool(name="wp", bufs=1) as w_pool,
            tc.tile_pool(name="xp", bufs=2) as x_pool,
            tc.tile_pool(name="fp", bufs=2) as f_pool,
            tc.tile_pool(name="sp", bufs=2) as s_pool,
            tc.tile_pool(name="op", bufs=2) as o_pool,
            tc.tile_pool(name="cp", bufs=1) as c_pool,
            tc.tile_pool(name="ps", bufs=1, space="PSUM") as ps_pool,
        ):
            consts = c_pool.tile([128, 1], F32, name="consts")
            nc.vector.memset(consts[:, 0:1], -0.5)

            # ---- hoisted: weights resident in SBUF across all reps ----
            wt = w_pool.tile([128, N_PLANES * O], F16, name="wt")
            wsrc = W_d[:].rearrange("(j p) o -> p j o", p=128)
            w3 = wt[:].rearrange("p (j o) -> p j o", j=N_PLANES)
            for j in range(N_PLANES):
                eng = nc.sync if j % 2 == 0 else nc.scalar
                eng.dma_start(w3[:, j, :], wsrc[:, j, :])

            state = {}

            def load_xt():
                """Issue the DMA for the NEXT body's xT slice."""
                xt = x_pool.tile([128, B], F32, tag="xt", name="xt")
                nc.sync.dma_start(xt[:, 0:BC], xT_d[:, 0:BC])
                nc.scalar.dma_start(xt[:, BC:], xT_d[:, BC:])
                return xt

            def produce(ft, xs, ch):
                """Write chunk ch's 9 feature planes (from xs) into ft.

                Only the first op reads the fp32 x slice; everything else
                reads the fp16 v plane (16-bit DVE mode, half the traffic).
                Equivalent to evaluating the spline at fp16-rounded x-0.5;
                the f'*dx error is ~4e-4 relative, inside budget.
                """
                def plane(p):
                    return ft[:, (ch * N_PLANES + p) * BC:
                              (ch * N_PLANES + p + 1) * BC]

                v = plane(0)
                nc.vector.tensor_scalar(v, xs, 0.5, None, ALU.subtract)
                nc.scalar.activation(plane(1), v, AF.Square)
                for t, cj in enumerate(LEFT + RIGHT):
                    gate = ALU.min if t < 3 else ALU.max
                    r = s_pool.tile([128, BC], F16, tag="r", name=f"r{ch}_{t}")
                    nc.vector.tensor_scalar(r[:], v, float(cj) - 0.5, 0.0,
                                            ALU.subtract, gate)
                    nc.scalar.activation(plane(2 + t), r[:], AF.Square)

            def mm_otile(ft, ot):
                """One o-tile: load each (j, ot) weight tile once, reuse it
                across all 4 batch chunks; 9-plane chains into 4 psum banks."""
                bank = (ot % 2) * 4
                ps = [ps_pool.tile([128, BC], F32, tag=f"ps{bank + c}",
                                   name=f"ps{ot}_{c}") for c in range(NCH)]
                for j in range(N_PLANES):
                    for c in range(NCH):
                        nc.tensor.matmul(
                            ps[c][:],
                            wt[:, j * O + ot * 128:j * O + (ot + 1) * 128],
                            ft[:, (c * N_PLANES + j) * BC:
                               (c * N_PLANES + j + 1) * BC],
                            start=(j == 0), stop=(j == N_PLANES - 1),
                        )
                otc = o_pool.tile([128, B], F16, tag=f"otc{ot % 2}",
                                  name=f"otc{ot}")
                for c in range(NCH):
                    dst = otc[:, c * BC:(c + 1) * BC]
                    if c % 2 == 0:
                        nc.vector.tensor_copy(dst, ps[c][:])
                    else:
                        nc.scalar.copy(dst, ps[c][:])
                eng = nc.sync if ot % 2 == 0 else nc.scalar
                eng.dma_start(yT_d[ot * 128:(ot + 1) * 128, :], otc[:])

            def body(_iv=None):
                xt = state['xt']
                nxt = load_xt()          # next body's x, DMA overlaps this body
                ft_cur = state['ft']
                ft_next = f_pool.tile([128, NCH * N_PLANES * BC], F16,
                                      tag="ft", name="ft")
                for ot in range(N_OT):
                    # produce next body's features (from next body's x),
                    # one chunk per pair of o-tiles, spread across the body
                    if ot % 2 == 0:
                        produce(ft_next, nxt[:, (ot // 2) * BC:
                                             (ot // 2 + 1) * BC], ot // 2)
                    mm_otile(ft_cur, ot)
                state['ft'] = ft_next
                state['xt'] = nxt

            def head():
                xt0 = x_pool.tile([128, B], F32, tag="xt", name="xt_h")
                nc.sync.dma_start(xt0[:, 0:BC], xT_d[:, 0:BC])
                nc.scalar.dma_start(xt0[:, BC:], xT_d[:, BC:])
                ft0 = f_pool.tile([128, NCH * N_PLANES * BC], F16, tag="ft",
                                  name="ft_h")
                for c in range(NCH):
                    produce(ft0, xt0[:, c * BC:(c + 1) * BC], c)
                state.update(xt=xt0, ft=ft0)

            head()
            if unroll > 1:
                assert reps == 1
                for _ in range(unroll):
                    body()
            elif reps == 1:
                body()
            else:
                assert reps % loop_unroll == 0
                with tc.For_i(0, reps // loop_unroll, 1) as iv:
                    for _ in range(loop_unroll):
                        body(iv)
    nc.compile()
    return nc


_cached_nc = None


def _get_nc():
    global _cached_nc
    if _cached_nc is None:
        _cached_nc = build_kernel(reps=1)
    return _cached_nc


def prepare_inputs(x, spline_parameters, w_b, w_s):
    """Host-side prep: returns (in_maps, bias, w_b, silu_sum) for the 8 cores."""
    x = np.ascontiguousarray(np.asarray(x, np.float32))
    P = np.asarray(spline_parameters, np.float32)
    w_b = float(np.asarray(w_b))
    W, bias = fold_weights(P, float(np.asarray(w_s)))
    xd = x.astype(np.float64)
    silu_sum = (xd / (1.0 + np.exp(-xd))).sum(axis=1)          # (B,)
    xT = np.ascontiguousarray(x.T)                             # (I, B)
    in_maps = []
    for c in range(N_CORES):
        sl = slice(c * I_LOC, (c + 1) * I_LOC)
        in_maps.append({
            "xT": np.ascontiguousarray(xT[sl, :]),
            "Wf": np.ascontiguousarray(
                W[:, sl, :].reshape(N_PLANES * I_LOC, O)),
        })
    return in_maps, bias, w_b, silu_sum


def kernel(x, spline_parameters, w_b, w_s):
    in_maps, bias, w_b, silu_sum = prepare_inputs(x, spline_parameters, w_b, w_s)
    nc = _get_nc()
    res = run_bass_kernel_spmd(nc, in_maps, core_ids=list(range(N_CORES)))
    acc = np.zeros((O, B), np.float64)
    for c in range(N_CORES):
        acc += res.results[c]["yT"].astype(np.float64)
    out = acc.T + bias[None, :]
    out += (w_b * silu_sum)[:, None]
    return out.astype(np.float32)
